# revision 1
# baseline (speedup 1.0000x reference)
"""Multi-head attention (B=64, N=577, E=1024, H=16) on 8 TRN2 NeuronCores.

Strategy: pure data-parallel over batch (8 batches/core), full weights on
every core. Per (batch, head): scores are computed directly in transposed
orientation S^T[nk, nq] so softmax needs no probability transpose; the
softmax denominator comes for free from a ones-column appended to V in the
P@V matmul; normalization + the final [b, n, e] permute happen on the host.

v5:
- all matmul operands in bf16 (same PE stream rate, half the SBUF/DVE
  traffic, fast weight load);
- x^T and W^T via regular matmul against identity (stationary = data,
  moving = identity): ~80ns pipelined vs ~300ns PE transpose-mode, and
  it counts as PE-busy for HAM clock warmth;
- exp merged into 3 ACT instructions per (head, nq-chunk) (the ACT
  engine's per-instruction overhead dominated the attention phase);
- full cross-batch pipelining: batch b+1's x^T/V/QK projections are
  emitted as PE filler inside batch b's ACT-bound attention phase;
- PSUM: scores 2x[128,1024] + PV 1x[65,512] + proj 3x[128,512] = 8 banks.
"""

import numpy as np

B, N, E, H, D = 64, 577, 1024, 16, 64
NCORES = 8
BL = B // NCORES            # batches per core
NP = 578                    # padded nq (even; pad col is zeroed)
EB = E // 128               # 8 e-blocks
NBL = [(i * 128, min(128, N - i * 128)) for i in range((N + 127) // 128)]
NGRP = [(0, 2), (2, 2), (4, 1)]      # nk-block groups for merged exp
CHUNKS = [(0, 290), (290, 288)]      # nq chunks (each fits a PSUM bank)

_CACHE = {}


def _build(cfg=None):
    cfg = cfg or {}
    ST = cfg.get("st", 2)
    PV = cfg.get("pv", 1)
    MM = cfg.get("mm", 3)
    QKB = cfg.get("qkb", 10)  # qt/kt rotation depth (pairs in flight)
    ESB = cfg.get("esb", 8)
    OVB = cfg.get("ovb", 4)
    NO_SC = cfg.get("no_sc", False)     # debug: proj only
    STQ = cfg.get("stq", 1)  # 1: all output stores on gpsimd SWDGE
    import concourse.mybir as mybir
    import concourse.tile as tile
    from concourse import bacc
    from concourse.masks import make_identity

    f32 = mybir.dt.float32
    bf16 = mybir.dt.bfloat16
    Exp = mybir.ActivationFunctionType.Exp

    nc = bacc.Bacc("TRN2", target_bir_lowering=False, debug=False,
                   num_devices=NCORES)
    x = nc.declare_dram_parameter("x", [BL, N, E], f32, isOutput=False)
    Wq = nc.declare_dram_parameter("Wq", [E, E], f32, isOutput=False)
    Wk = nc.declare_dram_parameter("Wk", [E, E], f32, isOutput=False)
    Wv = nc.declare_dram_parameter("Wv", [E, E], f32, isOutput=False)
    # output in [b, head, d(+sums row), n] layout; the host gather applies
    # the softmax normalization and the final [b, n, e] permute
    out = nc.declare_dram_parameter("out", [BL, H, D + 1, N], f32,
                                    isOutput=True)

    with tile.TileContext(nc) as tc:
        with (
            tc.tile_pool(name="sb", bufs=1) as sb,
            tc.tile_pool(name="ps", bufs=1, space="PSUM") as ps,
        ):
            ident = sb.tile([128, 128], bf16, tag="id", name="ident")
            make_identity(nc, ident[:])

            # ---- weights: W^T as one [e_in 128, ei 8, e_out 1024] tile
            # per matrix, so packed transposes land in ONE DVE copy ----
            wt = {wi: sb.tile([128, EB, E], bf16, tag=f"wt{wi}",
                              name=f"wt{wi}") for wi in range(3)}

            def stage_cvt(W_dram, r0, rsz):
                # DMA a 128-row block of a f32 DRAM matrix, convert to bf16
                ld = sb.tile([128, E], f32, tag="stage", bufs=3, name="ld")
                nc.sync.dma_start(out=ld[:rsz, :], in_=W_dram[r0:r0 + rsz, :])
                bft = sb.tile([128, E], bf16, tag="bfst", bufs=3, name="bf")
                nc.vector.tensor_copy(bft[:rsz, :], ld[:rsz, :])
                return bft

            def tr_pack4(dst3d, src, s0, tsz, step=None):
                # transpose 4 consecutive 128-col blocks of `src` via plain
                # matmuls (out = block.T @ I) packed into ONE bf16 psum
                # tile, evacuated by ONE strided DVE copy. Generator:
                # yields between PE ops so fillers can interleave.
                pt = ps.tile([128, 512], f32, tag="mm", bufs=MM, name="ptp")
                for s in range(4):
                    ei = s0 + s
                    nc.tensor.matmul(
                        pt[:, s * 128:s * 128 + tsz],
                        src[:tsz, ei * 128:(ei + 1) * 128],
                        ident[:tsz, :tsz], start=True, stop=True)
                    if step:
                        yield None
                nc.vector.tensor_copy(
                    dst3d, pt[:, :].rearrange(
                        "p (s c) -> p s c", s=4)[:, :, :tsz])
                if step:
                    yield None

            def drain_now(g):
                for _ in g:
                    pass

            def make_xtv(b):
                # x^T tiles for batch b + V->vext, as TWO generators:
                # gen_xtr (loads + transposes; touches no weights) and
                # gen_vproj (V projection; reads wt[2]). Split so batch 0's
                # x loads/transposes can overlap the Wv weight prologue.
                xt = sb.tile([128, EB, NP], bf16, tag="xt", bufs=2,
                             name="xt")
                vext = [sb.tile([128, H, D + 1], bf16, tag=f"vx_{nb}",
                                bufs=2, name=f"vx{nb}")
                        for nb in range(len(NBL))]

                def gen_xtr():
                    # dedicated x staging pools (NOT shared with W staging
                    # — sharing + deep bufs deadlocks the scheduler).
                    # All 5 block DMAs + converts are issued before the
                    # transposes so only the first DMA latency is exposed.
                    xbfs = []
                    for nb, (n0, nsz) in enumerate(NBL):
                        nc.gpsimd.memset(
                            vext[nb][:nsz, :, D:D + 1], 1.0)
                        ld = sb.tile([128, E], f32, tag="xstage", bufs=5,
                                     name="xld")
                        tsz = nsz
                        if n0 + nsz == N:  # zero row -> zero pad col 577
                            nc.gpsimd.memset(ld[64:66, :], 0.0)
                            tsz = nsz + 1
                        nc.sync.dma_start(out=ld[:nsz, :],
                                          in_=x[b, n0:n0 + nsz, :])
                        xbf = sb.tile([128, E], bf16, tag="xbfst", bufs=5,
                                      name="xbf")
                        nc.vector.tensor_copy(xbf[:tsz, :], ld[:tsz, :])
                        xbfs.append((xbf, n0, tsz))
                        yield None
                    for xbf, n0, tsz in xbfs:
                        for s0 in (0, 4):
                            yield from tr_pack4(
                                xt[:, s0:s0 + 4, n0:n0 + tsz],
                                xbf, s0, tsz, step=True)

                def gen_vproj():
                    for nb, (n0, nsz) in enumerate(NBL):
                        for ec in range(2):
                            pv = ps.tile([128, 512], f32, tag="mm", bufs=MM,
                                         name="pv")
                            for ei in range(EB):
                                nc.tensor.matmul(
                                    pv[:nsz, :], xt[:, ei, n0:n0 + nsz],
                                    wt[2][:, ei, ec * 512:(ec + 1) * 512],
                                    start=(ei == 0), stop=(ei == EB - 1))
                                yield None
                            nc.vector.tensor_copy(
                                vext[nb][:nsz, ec * 8:(ec + 1) * 8, 0:D],
                                pv[:nsz, :].rearrange("p (h d) -> p h d", d=D))
                            yield None

                return xt, vext, gen_xtr(), gen_vproj()

            def emit_qk(j, xt, with_w):
                qt = sb.tile([128, NP], bf16, tag="qt", bufs=QKB, name="qt")
                kt = sb.tile([128, NP], bf16, tag="kt", bufs=QKB, name="kt")

                def gen():
                    if with_w:
                        # transpose the eo=j block of Wq/Wk just-in-time
                        for wi, W in ((0, Wq), (1, Wk)):
                            wbf = stage_cvt(W, j * 128, 128)
                            for s0 in (0, 4):
                                yield from tr_pack4(
                                    wt[wi][:, s0:s0 + 4,
                                           j * 128:(j + 1) * 128],
                                    wbf, s0, 128, step=True)
                    for dst, wi in ((qt, 0), (kt, 1)):
                        for c0, cw in CHUNKS:
                            pq = ps.tile([128, 512], f32, tag="mm", bufs=MM,
                                         name="pq")
                            for ei in range(EB):
                                nc.tensor.matmul(
                                    pq[:, :cw],
                                    wt[wi][:, ei, j * 128:(j + 1) * 128],
                                    xt[:, ei, c0:c0 + cw],
                                    start=(ei == 0), stop=(ei == EB - 1))
                                yield None
                            nc.vector.tensor_copy(
                                dst[:, c0:c0 + cw], pq[:, :cw])
                            yield None

                return qt, kt, gen()

            # ---- filler machinery: one ordered queue of generators ----
            fillers = []

            def fill(n):
                for _ in range(n):
                    if not fillers:
                        return
                    for it in list(fillers):
                        if next(it, StopIteration) is StopIteration:
                            fillers.remove(it)
                        else:
                            break

            def drain(it, n=10 ** 6):
                for _ in range(n):
                    if next(it, StopIteration) is StopIteration:
                        return

            # proj_batch(b): xtv then all 8 qk pairs, as one sequential
            # generator. registry[(b, j)] is set just before pair j's
            # emission begins; attention(b, j) force-drains until present.
            registry = {}
            bundles = {}

            def proj_batch(b, pieces=None):
                xt, vext, gx, gv = pieces if pieces else make_xtv(b)
                bundles[b] = (xt, vext)

                def gen():
                    yield from gx
                    yield from gv
                    for j in range(H // 2):
                        qt, kt, qg = emit_qk(j, xt, with_w=(b == 0))
                        registry[b, j] = (qt, kt)
                        yield from qg
                        registry[b, j, "done"] = True
                return gen()

            # ---- weight prologue: Wv transposed first (V proj reads it),
            # overlapped with batch 0's x loads + transposes (gx0 touches
            # no weights, so interleaving its emission here is safe) ----
            pieces0 = make_xtv(0)
            gx0 = pieces0[2]
            fillers.append(gx0)
            for eo in range(EB):
                wbf = stage_cvt(Wv, eo * 128, 128)
                for s0 in (0, 4):
                    for _ in tr_pack4(
                            wt[2][:, s0:s0 + 4, eo * 128:(eo + 1) * 128],
                            wbf, s0, 128, step=True):
                        fill(1)
            if gx0 in fillers:
                fillers.remove(gx0)

            proj_gen = {0: proj_batch(0, pieces=pieces0)}
            fillers.append(proj_gen[0])

            for b in range(BL):
                if b + 1 < BL:
                    proj_gen[b + 1] = proj_batch(b + 1)
                    fillers.append(proj_gen[b + 1])
                xt_cur, vext = bundles[b]

                for j in range(H // 2):
                    # force pair (b, j) emission to completion
                    while (b, j, "done") not in registry:
                        drain(proj_gen[b], 1)
                    qt, kt = registry[b, j]
                    ovj = {}

                    for ci, (c0, cw) in enumerate(CHUNKS):
                        cwo = min(cw, N - c0)  # drop the nq pad column
                        if NO_SC:
                            for h in range(2):
                                ov = sb.tile([D + 1, 290], f32, tag="ov",
                                             bufs=4, name="ov")
                                nc.vector.tensor_copy(
                                    ov[:, :cw], qt[:D + 1, c0:c0 + cw])
                                fill(1)
                                eng = (nc.sync if (ci + h) % 2 == 0
                                       else nc.gpsimd)
                                eng.dma_start(
                                    out=out[b, 2 * j + h, :, c0:c0 + cwo],
                                    in_=ov[:, :cwo])
                                fill(1)
                            fill(8)
                            continue
                        pO = [ps.tile([D + 1, 512], f32, tag="pv", bufs=PV,
                                      name=f"pO{h}") for h in range(2)]
                        es_t = [[None] * len(NGRP) for _ in range(2)]
                        ngr = len(NGRP)
                        # all scores+exp groups first, then all PV groups:
                        # PV only ever waits on exps finished >=2 groups
                        # earlier, hiding the ACT latency behind PE work
                        for gi in range(ngr):
                            g0, gn = NGRP[gi]
                            for h in range(2):
                                pS = ps.tile([128, 1024], f32, tag="st",
                                             bufs=ST, name="pS")
                                for s in range(gn):
                                    k0, ksz = NBL[g0 + s]
                                    nc.tensor.matmul(
                                        pS[:ksz, s * 512:s * 512 + cw],
                                        kt[h * 64:h * 64 + 64,
                                           k0:k0 + ksz],
                                        qt[h * 64:h * 64 + 64,
                                           c0:c0 + cw],
                                        start=True, stop=True,
                                        tile_position=(h * 64, 0))
                                    fill(1)
                                e = sb.tile([128, 2, 290], bf16,
                                            tag="es", bufs=ESB, name="es")
                                ksz0 = NBL[g0][1]
                                src = pS[:ksz0, :].rearrange(
                                    "p (g c) -> p g c", g=2)
                                nc.scalar.activation(
                                    e[:ksz0, :gn, :cw],
                                    src[:, :gn, :cw], Exp, scale=0.125)
                                es_t[h][gi] = e
                                fill(1)
                        for gi in range(ngr):
                            pg0, pgn = NGRP[gi]
                            for h in range(2):
                                for s in range(pgn):
                                    k0, ksz = NBL[pg0 + s]
                                    nb = pg0 + s
                                    nc.tensor.matmul(
                                        pO[h][:, :cw],
                                        vext[nb][:ksz, 2 * j + h, :],
                                        es_t[h][gi][:ksz, s, :cw],
                                        start=(nb == 0),
                                        stop=(nb == len(NBL) - 1))
                                    fill(1)
                        # stage both chunks into one [65, NP] tile per
                        # (pair, head); ONE contiguous-row store per head
                        # after the second chunk (halves store-DMA count)
                        for h in range(2):
                            if ci == 0:
                                ovj[h] = sb.tile([D + 1, NP], f32,
                                                 tag="ov", bufs=OVB,
                                                 name="ov")
                            nc.vector.tensor_copy(
                                ovj[h][:, c0:c0 + cw], pO[h][:, :cw])
                            fill(1)
                            if ci == len(CHUNKS) - 1:
                                eng = (nc.gpsimd if STQ else
                                       (nc.sync if (j + h) % 2 == 0
                                        else nc.gpsimd))
                                eng.dma_start(
                                    out=out[b, 2 * j + h, :, :],
                                    in_=ovj[h][:, :N])
                                fill(1)
            fill(10 ** 6)

    nc.compile()
    return nc


def in_maps_for_bench(inputs):
    x = np.ascontiguousarray(np.asarray(inputs["x"], dtype=np.float32))
    Wq = np.ascontiguousarray(np.asarray(inputs["Wq"], dtype=np.float32))
    Wk = np.ascontiguousarray(np.asarray(inputs["Wk"], dtype=np.float32))
    Wv = np.ascontiguousarray(np.asarray(inputs["Wv"], dtype=np.float32))
    xs = x.reshape(NCORES, BL, N, E)
    return [
        {"x": np.ascontiguousarray(xs[i]), "Wq": Wq, "Wk": Wk, "Wv": Wv}
        for i in range(NCORES)
    ]


def kernel(x, Wq, Wk, Wv):
    from concourse.bass_utils import run_bass_kernel_spmd

    if "nc" not in _CACHE:
        _CACHE["nc"] = _build()
    nc = _CACHE["nc"]

    in_maps = in_maps_for_bench({"x": x, "Wq": Wq, "Wk": Wk, "Wv": Wv})
    res = run_bass_kernel_spmd(nc, in_maps, core_ids=list(range(NCORES)))
    # device emits [b, head, d(+sums), n]; normalize + permute on the host
    ot = np.concatenate([res.results[i]["out"] for i in range(NCORES)], axis=0)
    o = ot[:, :, :D, :] / ot[:, :, D:D + 1, :]
    return np.ascontiguousarray(
        o.transpose(0, 3, 1, 2).reshape(B, N, E).astype(np.float32))

